# revision 9
# baseline (speedup 1.0000x reference)
"""Trainium2 Bass kernel for nn_AttentionHead (B=4, S=4096, H=1024, D=64).

Reference computation (note the unusual K-first ordering):
    K = x @ Wk.T; Q = x @ Wq.T; V = x @ Wv.T            [B,S,D]
    scores[b,i,j] = (K[b,i] . Q[b,j]) / sqrt(D)         [B,S,S]
    scores[:, :, j] = -1e12 where mask[:, j] == 0
    out = softmax(scores, axis=2) @ V                   [B,S,D]

Sharding: 8 cores = 4 batches x 2 key-row chunks of 2048. Each core gets a
batch's x ROLLED so its own key rows are always rows [0, 2048) — the SPMD
program is identical across cores. Softmax runs over the full (rolled) query
axis on every core, so rolling is correctness-neutral.

Per-core pipeline (bf16 matmuls, fp32 accumulation):
  0. Warmup matmuls on junk data keep the PE HAM clock-gate at 8/8 while
     x is DMA-cast to bf16 (DRAM->DRAM) and xbar-DMA-transposed into SBUF
     on both HWDGE queues.
  1. Projections on PE: one [Wq|Wv] stationary gives Q^T (rows 0:64) and
     V^T (rows 64:128) per 512-col block; K^T separately over own rows;
     V^T -> V via PE transposes. V gets a ones column for the softmax denom.
  2. Two passes over query-tile loop, one per 1024-wide i-half: scores^T =
     Q^T_t.T @ K^T on PE; exp(0.125*s + maskbias[j]) on ACT (mask folded
     into the per-partition bias; masked queries underflow to exactly 0);
     PE accumulates V'_t.T @ P^T_t into out'^T [65, 1024] PSUM (rows 0:64
     numerator^T, row 64 denominator; one bank per 512-col window).
  3. Per-pass finale: PE-transpose out'^T via identity matmul, then
     out = numerator * reciprocal(denominator) on DVE; one DMA store.
"""

import numpy as np

B, S, H, D = 4, 4096, 1024, 64
N_CORES = 8
SC = S // 2  # key rows per core
HC = H // 128  # contraction chunks
JT = S // 128  # query tiles
NEG = -30000.0
N_WARM = 48

_CACHE = {}


def _build():
    import concourse.bass as bass
    import concourse.tile as tile
    from concourse import bacc, mybir

    dt = mybir.dt
    AF = mybir.ActivationFunctionType

    nc = bacc.Bacc(
        "TRN2", target_bir_lowering=False, debug=False, num_devices=N_CORES
    )
    x = nc.dram_tensor("x", [S, H], dt.float32, kind="ExternalInput").ap()
    wqv = nc.dram_tensor("wqv", [H, 2 * D], dt.float32, kind="ExternalInput").ap()
    wkt = nc.dram_tensor("wkt", [H, D], dt.float32, kind="ExternalInput").ap()
    mb = nc.dram_tensor("mb", [128, JT], dt.float32, kind="ExternalInput").ap()
    ident = nc.dram_tensor("ident", [128, 128], dt.float32, kind="ExternalInput").ap()
    out = nc.dram_tensor("out", [SC, D], dt.float32, kind="ExternalOutput").ap()

    xbf = nc.dram_tensor("xbf", [S, H], dt.bfloat16).ap()

    with tile.TileContext(nc) as tc:
        with tc.tile_pool(name="persist", bufs=1) as persist:
            qt = persist.tile([128, S], dt.bfloat16)  # rows 0:64 = Q^T
            kt = persist.tile([128, SC], dt.bfloat16)  # rows 0:64 = K^T
            vtsb = persist.tile([128, S], dt.bfloat16)  # rows 64:128 = V^T
            vp = persist.tile([128, JT, D + 1], dt.bfloat16)
            mb_sb = persist.tile([128, JT], dt.float32)
            id_f32 = persist.tile([128, 128], dt.float32)
            id_bf = persist.tile([128, 128], dt.bfloat16)
            wtile = persist.tile([128, 512], dt.bfloat16)

            nc.sync.dma_start(mb_sb[:], mb[:])
            nc.sync.dma_start(id_f32[:], ident[:])
            nc.gpsimd.dma_start(id_bf[:], ident[:])
            nc.vector.memset(vp[:, :, D], 1.0)
            nc.vector.memset(wtile[:], 0.0)

            with (
                tc.tile_pool(name="xpool", bufs=1) as xpool,
                tc.tile_pool(name="pwarm", bufs=1, space="PSUM") as pwarm,
            ):
                xT = xpool.tile([128, HC, S], dt.bfloat16)
                wqv_sb = xpool.tile([128, HC, 2 * D], dt.bfloat16)
                wk_sb = xpool.tile([128, HC, D], dt.bfloat16)

                nc.gpsimd.dma_start(wqv_sb[:], wqv.rearrange("(c p) d -> p c d", p=128))
                nc.gpsimd.dma_start(wk_sb[:], wkt.rearrange("(c p) d -> p c d", p=128))

                # PE warmup while the transposes stream in
                pw = pwarm.tile([128, 512], dt.float32)
                for _ in range(N_WARM):
                    nc.tensor.matmul(
                        pw[:], wtile[:, 0:128], wtile[:], start=True, stop=True
                    )
                # preload the exp table
                dummy = xpool.tile([128, 1], dt.float32)
                nc.scalar.activation(dummy[:], wtile[:, 0:1], AF.Exp)

                # x^T: cast x to bf16 (SWDGE DRAM->DRAM), then xbar-transpose
                # into SBUF, alternating between the two HWDGE queues.
                NQ = 4
                QS = S // NQ
                for q in range(NQ):
                    nc.gpsimd.dma_start(
                        xbf[q * QS : (q + 1) * QS, :], x[q * QS : (q + 1) * QS, :]
                    )
                    for hc in range(HC):
                        eng = nc.sync
                        eng.dma_start(
                            xT[:, hc, q * QS : (q + 1) * QS],
                            xbf[q * QS : (q + 1) * QS, 128 * hc : 128 * (hc + 1)],
                            transpose=True,
                        )

                # --- projections ---
                with (
                    tc.tile_pool(name="pprojj", bufs=3, space="PSUM") as pproj,
                    tc.tile_pool(name="pprojk", bufs=2, space="PSUM") as pprojk,
                    tc.tile_pool(name="pprojv", bufs=2, space="PSUM") as pprojv,
                ):
                    for sb in range(S // 512):  # [Q^T; V^T] over all 4096 queries
                        ps = pproj.tile([128, 512], dt.float32, tag="pj")
                        for hc in range(HC):
                            nc.tensor.matmul(
                                ps[:],
                                wqv_sb[:, hc, :],
                                xT[:, hc, 512 * sb : 512 * (sb + 1)],
                                start=(hc == 0),
                                stop=(hc == HC - 1),
                            )
                        nc.vector.tensor_copy(
                            qt[0:64, 512 * sb : 512 * (sb + 1)], ps[0:64, :]
                        )
                        nc.vector.tensor_copy(
                            vtsb[64:128, 512 * sb : 512 * (sb + 1)], ps[64:128, :]
                        )
                    for sb in range(SC // 512):  # K^T over own 2048 key rows
                        ps = pprojk.tile([64, 512], dt.float32, tag="pk")
                        for hc in range(HC):
                            nc.tensor.matmul(
                                ps[:],
                                wk_sb[:, hc, :],
                                xT[:, hc, 512 * sb : 512 * (sb + 1)],
                                start=(hc == 0),
                                stop=(hc == HC - 1),
                            )
                        nc.vector.tensor_copy(
                            kt[0:64, 512 * sb : 512 * (sb + 1)], ps[:]
                        )
                    for st in range(JT):  # V^T -> V via PE transpose
                        pvt = pprojv.tile([128, D], dt.bfloat16, tag="pv")
                        nc.tensor.transpose(
                            pvt[:],
                            vtsb[64:128, 128 * st : 128 * (st + 1)],
                            id_bf[64:128, 64:128],
                        )
                        nc.vector.tensor_copy(vp[:, st, 0:D], pvt[:])

            # --- main attention loop: two passes over 1024-wide i-halves ---
            with (
                tc.tile_pool(name="pacc", bufs=2, space="PSUM") as pacc,
                tc.tile_pool(name="psco", bufs=2, space="PSUM") as psco,
                tc.tile_pool(name="ptile", bufs=4) as ptile,
                tc.tile_pool(name="accs", bufs=2) as accs,
                tc.tile_pool(name="fin", bufs=2) as fin,
            ):
                oall = fin.tile([128, 16, D], dt.float32, tag="oall")
                for ih in range(2):
                    acc = pacc.tile([D + 1, 1024], dt.float32, tag="acc")
                    for t in range(JT):
                        ps = psco.tile([128, 1024], dt.float32)
                        for nb in range(2):
                            nc.tensor.matmul(
                                ps[:, 512 * nb : 512 * (nb + 1)],
                                qt[0:64, 128 * t : 128 * (t + 1)],
                                kt[
                                    0:64,
                                    1024 * ih + 512 * nb : 1024 * ih + 512 * (nb + 1),
                                ],
                                start=True,
                                stop=True,
                            )
                        pt = ptile.tile([128, 1024], dt.bfloat16)
                        nc.scalar.activation(
                            pt[:], ps[:], AF.Exp, bias=mb_sb[:, t : t + 1], scale=0.125
                        )
                        for nb in range(2):
                            nc.tensor.matmul(
                                acc[:, 512 * nb : 512 * (nb + 1)],
                                vp[:, t, :],
                                pt[:, 512 * nb : 512 * (nb + 1)],
                                start=(t == 0),
                                stop=(t == JT - 1),
                            )

                    # per-pass finale: transpose + normalize
                    acc_sb = accs.tile([D + 1, 1024], dt.float32, tag="accs")
                    nc.vector.tensor_copy(acc_sb[:], acc[:])
                    for k in range(8):
                        po = pacc.tile([128, D + 1], dt.float32, tag="acc")
                        nc.tensor.transpose(
                            po[:],
                            acc_sb[:, 128 * k : 128 * (k + 1)],
                            id_f32[0 : D + 1, 0 : D + 1],
                        )
                        rc = fin.tile([128, 1], dt.float32, tag="rc")
                        nc.vector.reciprocal(rc[:], po[:, D : D + 1])
                        nc.vector.tensor_scalar_mul(
                            oall[:, 8 * ih + k, :], po[:, 0:D], rc[:]
                        )
                nc.sync.dma_start(out.rearrange("(k p) d -> p k d", p=128), oall[:])

    nc.compile()
    return nc


def _in_maps(x, mask, Wk, Wq, Wv):
    wqv = np.ascontiguousarray(
        np.concatenate([Wq.T, Wv.T], axis=1), dtype=np.float32
    )
    wkt = np.ascontiguousarray(Wk.T, dtype=np.float32)
    ident = np.eye(128, dtype=np.float32)
    maps = []
    for c in range(N_CORES):
        b, half = c // 2, c % 2
        i0 = half * SC
        xr = np.ascontiguousarray(np.roll(x[b], -i0, axis=0))
        mr = np.roll(mask[b], -i0)
        mbv = np.where(mr == 0, np.float32(NEG), np.float32(0.0)).astype(np.float32)
        mbt = np.ascontiguousarray(mbv.reshape(JT, 128).T)  # [128, JT], j = 128*t + p
        maps.append({"x": xr, "wqv": wqv, "wkt": wkt, "mb": mbt, "ident": ident})
    return maps


def kernel(x, mask, Wk, Wq, Wv):
    from concourse.bass_utils import run_bass_kernel_spmd

    if "nc" not in _CACHE:
        _CACHE["nc"] = _build()
    nc = _CACHE["nc"]
    maps = _in_maps(x, mask, Wk, Wq, Wv)
    br = run_bass_kernel_spmd(nc, maps, list(range(N_CORES)))
    out = np.empty((B, S, D), dtype=np.float32)
    for c in range(N_CORES):
        b, half = c // 2, c % 2
        out[b, half * SC : (half + 1) * SC, :] = br.results[c]["out"]
    return out
